# revision 22
# baseline (speedup 1.0000x reference)
"""Causal single-head attention for B=8, T=2048, D=1024, HS=64 on 8 TRN2 cores.

Data-parallel over batch: core i computes batch element i entirely locally.

Per-core pipeline (fp16 compute, fp32 accumulate):
  1. cast-DMA x -> SBUF fp16 tiles; PE-transpose to xT (d on partitions)
  2. qT/kT = W.T @ xT  [64, 2048]; v natural [2048, 64] + ones col -> v_aug
  3. per k-chunk: S^T[k, q] = kT-slice.T @ qT (PSUM fp32), additive causal
     mask on the diagonal block, P^T = exp(scale * S^T) on ACT (-> fp16),
     out^T[65, q] += v_aug.T @ P^T  (row 64 = softmax denominator)
  4. transpose out^T back, divide by denominator, DMA out (fp32)

No max-subtraction in softmax: scale = 1/sqrt(2048) keeps |scale*S| < ~2.

This walrus build supports at most ONE sync wait / sync update per
instruction; Tile emits more, so we hoist extras onto InstNoOp neighbours
(see _patch_tile_for_single_wait_walrus).
"""

import math
import os

import numpy as np

import concourse.bass as bass
import concourse.mybir as mybir
import concourse.tile as tile
from concourse.bass_utils import run_bass_kernel_spmd
from concourse.vector_clock import ScopedClock
from contextlib import ExitStack

F32 = mybir.dt.float32
F16 = mybir.dt.float16

B, T, D, HS = 8, 2048, 1024, 64
NT = T // 128  # 16 row tiles
NC = D // 128  # 8 contraction chunks
SCALE = 1.0 / math.sqrt(2048.0)
NEG = -1.0e9

_patched = False


def _patch_tile_for_single_wait_walrus():
    """Split multi-wait / multi-update instructions into single-sync ones."""
    global _patched
    if _patched:
        return
    _patched = True

    orig_add = tile.TileContext._add_instruction

    def patched_add(self, inst):
        si = getattr(inst, "sync_info", None)
        if si is not None and (len(si.on_wait) > 1 or len(si.on_update) > 1):
            waits = list(si.on_wait)
            updates = list(si.on_update)
            for w in waits[:-1]:
                nop = mybir.InstNoOp(
                    name=self.nc.get_next_instruction_name(),
                    engine=inst.engine,
                    sync_info=mybir.SyncInfo(on_wait=[w], on_update=[]),
                    bass_nofuse=True,
                )
                orig_add(self, nop)
            inst.sync_info = mybir.SyncInfo(on_wait=waits[-1:], on_update=updates[:1])
            orig_add(self, inst)
            for u in updates[1:]:
                nop = mybir.InstNoOp(
                    name=self.nc.get_next_instruction_name(),
                    engine=inst.engine,
                    sync_info=mybir.SyncInfo(on_wait=[], on_update=[u]),
                    bass_nofuse=True,
                )
                orig_add(self, nop)
            return
        orig_add(self, inst)

    tile.TileContext._add_instruction = patched_add

    def patched_drain(self, tick_clock, wait_clock):
        probe = self.nc.sync.nop()
        wait_clock.add_sem_waits(
            probe.ins, ScopedClock({None: tick_clock.global_clock})
        )
        si = probe.ins.sync_info
        waits = list(si.on_wait) if si is not None else []
        if si is not None:
            probe.ins.sync_info = mybir.SyncInfo(
                on_wait=[], on_update=list(si.on_update)
            )
        for w in waits:
            n = self.nc.sync.nop()
            n.ins.sync_info = mybir.SyncInfo(on_wait=[w], on_update=[])
        self.nc.sync.drain()
        self.nc.all_engine_barrier()
        popped = self.nc._tile_sem_poison_stack.pop()
        assert popped is self._sem_poison
        self.nc.clear_and_free_semaphores(list(self.sems.allocated().values()))

    tile.TileContext._drain_and_barrier = patched_drain


def build():
    nc = bass.Bass("TRN2", target_bir_lowering=False, debug=False)
    x = nc.dram_tensor("x", [T, D], F32, kind="ExternalInput").ap()
    wq = nc.dram_tensor("wq", [D, HS], F32, kind="ExternalInput").ap()
    wk = nc.dram_tensor("wk", [D, HS], F32, kind="ExternalInput").ap()
    wv = nc.dram_tensor("wv", [D, HS], F32, kind="ExternalInput").ap()
    id16 = nc.dram_tensor("id16", [128, 128], F16, kind="ExternalInput").ap()
    id32 = nc.dram_tensor("id32", [65, 65], F32, kind="ExternalInput").ap()
    trimask = nc.dram_tensor("trimask", [128, 128], mybir.dt.uint16, kind="ExternalInput").ap()
    out = nc.dram_tensor("out", [T, HS], F32, kind="ExternalOutput").ap()

    with tile.TileContext(nc) as tc, ExitStack() as ctx:
        sb = ctx.enter_context(tc.tile_pool(name="sb", bufs=1))
        sb2 = ctx.enter_context(tc.tile_pool(name="sb2", bufs=4))
        pt_pool = ctx.enter_context(tc.tile_pool(name="ptp", bufs=6))
        # one shared PSUM pool: 3 slots x 4KB (2 banks) + oT accumulators
        wk_pool = ctx.enter_context(tc.tile_pool(name="work", bufs=3, space="PSUM"))
        o_pool = ctx.enter_context(tc.tile_pool(name="pout", bufs=1, space="PSUM"))

        def wtile(shape, dtype, name):
            return wk_pool.tile(shape, dtype, tag="work", name=name)

        # ---- identities via HWDGE (fast, independent of SWDGE queue)
        ident16 = sb.tile([128, 128], F16, tag="id16")
        nc.sync.dma_start(ident16[:], id16)
        ident32 = sb.tile([65, 65], F32, tag="id32")
        nc.sync.dma_start(ident32[:], id32)
        tri_sb = sb.tile([128, 128], mybir.dt.uint16, tag="tri")
        nc.sync.dma_start(tri_sb[:], trimask)
        zero_sb = sb.tile([128, 128], F16, tag="zeros")
        nc.gpsimd.memset(zero_sb[:], 0.0)
        vaug = sb.tile([128, NT * 72], F16, tag="vaug")
        nc.gpsimd.memset(vaug[:], 1.0)
        # preload the exp table set long before the first real exp
        warm = sb.tile([1, 2], F32, tag="warm")
        nc.scalar.activation(
            warm[:], ident32[0:1, 0:2], mybir.ActivationFunctionType.Exp
        )

        # ---- x tiles 0-3 first (transposes gate on them), then W (q/k
        # projections need it by ~t+8us), then the rest of x, ~4 in flight
        x16 = [
            sb.tile([128, D], F16, tag=f"x16_{t}", name=f"x16_{t}")
            for t in range(NT)
        ]
        x_dmas = []
        for t in range(4):
            x_dmas.append(nc.gpsimd.dma_start(x16[t][:], x[128 * t : 128 * (t + 1), :]))
        w16 = {}
        for name, w in (("q", wq), ("k", wk), ("v", wv)):
            w16[name] = sb.tile([128, NC * HS], F16, tag=f"w{name}", name=f"w16{name}")
            nc.gpsimd.dma_start(
                w16[name][:].rearrange("p (c h) -> p c h", c=NC),
                w.rearrange("(c p) h -> p c h", p=128),
            )
        for t in range(4, NT):
            dma = nc.gpsimd.dma_start(x16[t][:], x[128 * t : 128 * (t + 1), :])
            bass._add_dep_helper(
                dma.ins, x_dmas[t - 4].ins, sync=True, reason="dma throttle"
            )
            x_dmas.append(dma)

        xT = sb.tile([128, NC * T], F16, tag="xT")
        xT3 = xT[:].rearrange("p (c t) -> p c t", c=NC)
        qT = sb.tile([64, T], F16, tag="qT")
        kT = sb.tile([64, T], F16, tag="kT")
        vaug3 = vaug[:].rearrange("p (t w) -> p t w", t=NT)
        out2 = out.rearrange("(g p) h -> p g h", p=128)

        def emit_transpose_group(ts):
            for t in ts:
                ptr = wtile([128, 1024], F16, f"ptr_{t}")
                for c in range(NC):
                    nc.tensor.transpose(
                        ptr[:, 128 * c : 128 * (c + 1)],
                        x16[t][:, 128 * c : 128 * (c + 1)],
                        ident16[:],
                    )
                # ACT is idle before the exp stream starts; DVE gets busier
                src = ptr[:].rearrange("p (c u) -> p c u", c=NC)
                dst = xT3[:, :, 128 * t : 128 * (t + 1)]
                if t < 8:
                    nc.scalar.copy(dst, src)
                else:
                    nc.vector.tensor_copy(dst, src)

        def emit_qk_slice(s):
            # q -> PSUM rows 0-63 (col group 0) and k -> rows 64-127 (col
            # group 64) run CONCURRENTLY on the PE (they share the moving
            # operand). kT is then shuffled down to partitions 0-63 by DMA
            # so S matmuls see both operands at base partition 0.
            pp = wtile([128, 512], F32, f"pqk_{s}")
            for c in range(NC):
                rhs = xT[:, T * c + 512 * s : T * c + 512 * (s + 1)]
                nc.tensor.matmul(
                    pp[0:64, :],
                    w16["q"][:, HS * c : HS * (c + 1)],
                    rhs,
                    start=(c == 0),
                    stop=(c == NC - 1),
                )
                nc.tensor.matmul(
                    pp[64:128, :],
                    w16["k"][:, HS * c : HS * (c + 1)],
                    rhs,
                    start=(c == 0),
                    stop=(c == NC - 1),
                )
            qk_sb = sb2.tile([128, 512], F16, tag="qk_sb", name=f"qksb_{s}")
            nc.vector.tensor_copy(qk_sb[:], pp[:])
            nc.vector.tensor_copy(qT[:, 512 * s : 512 * (s + 1)], qk_sb[0:64, :])
            nc.sync.dma_start(kT[:, 512 * s : 512 * (s + 1)], qk_sb[64:128, :])

        def emit_v_group(g):
            pv = wtile([128, 512], F32, f"pv_{g}")
            for ti in range(8):
                t = 8 * g + ti
                for c in range(NC):
                    nc.tensor.matmul(
                        pv[:, 64 * ti : 64 * (ti + 1)],
                        xT[:, T * c + 128 * t : T * c + 128 * (t + 1)],
                        w16["v"][:, HS * c : HS * (c + 1)],
                        start=(c == 0),
                        stop=(c == NC - 1),
                    )
            nc.vector.tensor_copy(
                vaug3[:, 8 * g : 8 * (g + 1), 0:64],
                pv[:].rearrange("p (t h) -> p t h", t=8),
            )

        def emit_bank_tail(h, b, oTb):
            """Normalize + store q rows [1024h+512b, +512) as soon as that
            bank's PV accumulation is complete. oTb = [65, 1024] accumulator
            for the half; bank b reads its 512-slice."""
            oT_sb = sb2.tile([65, 512], F32, tag="oT_sb", name=f"oTsb_{h}_{b}")
            nc.vector.tensor_copy(oT_sb[:], oTb[:, 512 * b : 512 * (b + 1)])
            r32 = sb2.tile([128, 4], F32, tag="r32", name=f"r32_{h}_{b}")
            out_sb = sb2.tile([128, 256], F32, tag="out_sb", name=f"osb_{h}_{b}")
            otr = wtile([128, 512], F32, f"otr_{h}_{b}")
            for j in range(4):
                nc.tensor.transpose(
                    otr[:, 128 * j : 128 * j + 65],
                    oT_sb[:, 128 * j : 128 * (j + 1)],
                    ident32[:],
                )
                nc.vector.reciprocal(
                    r32[:, j : j + 1], otr[:, 128 * j + 64 : 128 * j + 65]
                )
                nc.vector.tensor_scalar_mul(
                    out_sb[:, 64 * j : 64 * (j + 1)],
                    otr[:, 128 * j : 128 * j + 64],
                    r32[:, j : j + 1],
                )
            g0 = 8 * h + 4 * b
            nc.sync.dma_start(
                out2[:, g0 : g0 + 4, :],
                out_sb[:].rearrange("p (g w) -> p g w", g=4),
            )

        class Attn:
            """Attention for one q-half in k-chunk units: S^T [128, 1024]
            -> exp -> PV, PV lagging LAG k-chunks behind so the PE does not
            stall on the ACT exp stream. `filler` emits independent PE work
            (next phase's transposes) between units to fill residual gaps."""

            LAG = 3

            def __init__(self, h):
                self.h = h
                self.n_kc = 8 * h + 8
                self.last = [8 * h + 4 - 1, 8 * h + 8 - 1]
                self.oT = o_pool.tile([65, 1024], F32, tag="pout", name=f"oT_{h}")
                self.pending = []

            def emit_s_exp(self, kc):
                h = self.h
                q0 = 1024 * h
                qlo = max(0, 128 * kc - q0)
                sps = wtile([128, 1024], F32, f"s_{h}_{kc}")
                segs = [(qlo, 512), (512, 1024)] if qlo < 512 else [(qlo, 1024)]
                for a, b in segs:
                    nc.tensor.matmul(
                        sps[:, a:b],
                        kT[:, 128 * kc : 128 * (kc + 1)],
                        qT[:, q0 + a : q0 + b],
                        start=True,
                        stop=True,
                    )
                pT = pt_pool.tile([128, 1024], F16, tag="pT", name=f"pT_{h}_{kc}")
                nc.scalar.activation(
                    pT[:, qlo:1024],
                    sps[:, qlo:1024],
                    mybir.ActivationFunctionType.Exp,
                    scale=SCALE,
                )
                if kc >= 8 * h:
                    # zero P^T[k, q] where q < k inside the diagonal block
                    nc.vector.copy_predicated(
                        pT[:, qlo : qlo + 128], tri_sb[:], zero_sb[:]
                    )
                return qlo, pT

            def emit_pv(self, kc, qlo, pT):
                for b in range(2):
                    a0 = max(qlo, 512 * b)
                    b0 = 512 * (b + 1)
                    if a0 >= b0:
                        continue
                    nc.tensor.matmul(
                        self.oT[:, a0:b0],
                        vaug3[:, kc, 0:65],
                        pT[:, a0:b0],
                        start=(kc == 0),
                        stop=(kc == self.last[b]),
                    )
                    if kc == self.last[b]:
                        emit_bank_tail(self.h, b, self.oT)

            def run(self, kcs, filler=None, flush=False):
                for kc in kcs:
                    self.pending.append((kc, self.emit_s_exp(kc)))
                    if filler is not None:
                        filler(kc)
                    if len(self.pending) > self.LAG:
                        pkc, (pqlo, ppT) = self.pending.pop(0)
                        self.emit_pv(pkc, pqlo, ppT)
                if flush:
                    for pkc, (pqlo, ppT) in self.pending:
                        self.emit_pv(pkc, pqlo, ppT)
                    self.pending = []

        # ---- interleaved schedule: h0 attention as soon as its inputs
        # exist; ALL remaining phase-1 work (transposes t8-15, qk slices
        # 2-3) rides inside h0's stream as PE filler so h1 starts early.
        emit_transpose_group(range(0, 4))
        emit_qk_slice(0)
        emit_transpose_group(range(4, 8))
        emit_qk_slice(1)
        a0 = Attn(0)
        a0.run(range(0, 2))
        emit_v_group(0)
        fillers = {
            2: lambda: emit_transpose_group([8, 9]),
            3: lambda: emit_transpose_group([10, 11]),
            4: lambda: emit_transpose_group([12, 13]),
            5: lambda: emit_transpose_group([14, 15]),
            6: lambda: emit_qk_slice(2),
            7: lambda: emit_qk_slice(3),
        }
        a0.run(range(2, 8), filler=lambda kc: fillers[kc]())
        a0.run([], flush=True)
        emit_v_group(1)
        a1 = Attn(1)
        a1.run(range(0, 16), flush=True)

    return nc


_nc_cache = None


def _get_nc():
    global _nc_cache
    if _nc_cache is None:
        _patch_tile_for_single_wait_walrus()
        _nc_cache = build()
    return _nc_cache


def _make_in_maps(x, Wq, Wk, Wv):
    id16 = np.eye(128, dtype=np.float16)
    id32 = np.eye(65, dtype=np.float32)
    # S^T layout [k(part), q(free)]: invalid where q < k
    tri = (np.arange(128)[None, :] < np.arange(128)[:, None]).astype(np.uint16)
    x = np.ascontiguousarray(np.asarray(x, dtype=np.float32))
    Wq = np.ascontiguousarray(np.asarray(Wq, dtype=np.float32))
    Wk = np.ascontiguousarray(np.asarray(Wk, dtype=np.float32))
    Wv = np.ascontiguousarray(np.asarray(Wv, dtype=np.float32))
    return [
        {
            "x": x[i],
            "wq": Wq,
            "wk": Wk,
            "wv": Wv,
            "id16": id16,
            "id32": id32,
            "trimask": tri,
        }
        for i in range(B)
    ]


def run(x, Wq, Wk, Wv, trace=False):
    nc = _get_nc()
    in_maps = _make_in_maps(x, Wq, Wk, Wv)
    res = run_bass_kernel_spmd(nc, in_maps, core_ids=list(range(B)), trace=trace)
    out = np.stack([res.results[i]["out"] for i in range(B)]).astype(np.float32)
    return out, res


def kernel(x, Wq, Wk, Wv):
    out, _ = run(x, Wq, Wk, Wv, trace=bool(os.environ.get("KERNEL_TRACE")))
    return out
